# revision 17
# baseline (speedup 1.0000x reference)
"""Trainium2 Bass kernel for nn_E2EGuidedFilter (guided filter, r=8, eps=0.01).

Full inputs x, y: (8, 3, 1024, 1024) fp32. Data-parallel: one image per
NeuronCore (8 cores).

Per-core pipeline (per channel, H=W=1024, 8 partition-blocks of 128), v10:
  - host-prepped f16 inputs: xc = x-0.5, yc = y-0.5, xx = xc^2 (layout A)
    and xcbq = transpose(xc) * qw[w] (layout B, output-normalization
    prefolded). Output is f16 layout B; host transposes/casts back.
  - stage 1 box means: H-filter and W-filter are banded matmuls on the
    TensorEngine (qh resp. qw folded into the weights); the PSUM->SBUF
    f16 mid evacuations run on the Activation engine (Copy, with +eps
    bias folded into the xx evac).
  - pointwise (layout A) split by the cost model: ACT consumes PSUM
    (mx/my/u), DVE does the f16 2x math (t1/num/s2/den/recip/av), GpSimd
    (idle otherwise; cannot touch PSUM) does j1 and the fused
    bv = (my+0.5)-j1 scalar_tensor_tensor.
  - stage 2 box filter of a,b: W-direction via tensor_tensor_scan on
    GpSimd (zero-padded buffers), H-direction via banded matmul (qh
    folded) -> layout B; final out = (z2b*qw + z2a*xcbq) via DVE stt.
"""

import os
import sys

import numpy as np

for _p in ("/opt/trn_rl_repo", "/root/.axon_site/_ro/trn_rl_repo"):
    if os.path.isdir(_p) and _p not in sys.path:
        sys.path.append(_p)

R = 8
EPS = 0.01
H = W = 1024
PB = H // 128  # 8 partition blocks
C = 3
NCORES = 8
SCAN_LEN = W + R  # 1032
PAD0 = 18  # interior offset in scan input buffers
CH_PAD = PAD0 + W + 14  # 1056: padded chunk stride
CH_S = SCAN_LEN  # 1032: scan-output / mid chunk stride

_CACHE = {}


def _counts():
    i = np.arange(H)
    return (np.minimum(i + R, H - 1) - np.maximum(i - R, 0) + 1).astype(np.float64)


def _host_consts():
    qh = (1.0 / _counts()).astype(np.float32)

    def band_block(c, lo, n):
        Wt = np.zeros((128, n), np.float32)
        for j in range(n):
            hp = lo + j
            k0 = max(0, hp - R - 128 * c)
            k1 = min(127, hp + R - 128 * c)
            if k0 <= k1:
                Wt[k0 : k1 + 1, j] = qh[hp]
        return Wt

    W0 = band_block(0, 0, 136)
    Wi = band_block(1, 120, 144)
    W7 = band_block(7, 888, 136)
    wq = np.concatenate([W0, Wi, W7], axis=1).astype(np.float16)  # [128,416]
    qv = qh.reshape(PB, 128).T.copy().astype(np.float32)  # [128,8]
    return wq, qv


def _mm_windows():
    halves = [[], []]
    for c in range(PB):
        lo = max(0, 128 * c - 8)
        hi = min(1024, 128 * c + 136)
        if c == 0:
            wt, wbase = "e0", 0
        elif c == PB - 1:
            wt, wbase = "e7", 888
        else:
            wt, wbase = "int", 128 * c - 8
        for hf in (0, 1):
            blo, bhi = 512 * hf, 512 * hf + 512
            s, e = max(lo, blo), min(hi, bhi)
            if s < e:
                halves[hf].append((c, s, e, wt, s - wbase, e - wbase))
    return halves


_HALVES = _mm_windows()


def _split_multi_waits(nc, mybir):
    """This container's walrus supports 1 sync wait per instruction (2 for
    EventSemaphore); Tile emits more. Move excess waits onto NoOps inserted
    just before the instruction on the same engine."""
    uid = [0]
    for f in nc.m.functions:
        for bb in f.blocks:
            out = []
            changed = False
            for inst in bb.instructions:
                si = inst.sync_info
                waits = list(si.on_wait) if si and si.on_wait else []
                cap = 2 if type(inst).__name__ == "InstEventSemaphore" else 1
                if len(waits) > cap:
                    for w in waits[:-cap]:
                        uid[0] += 1
                        nop = mybir.InstNoOp(name=f"wsplit-{uid[0]}", ins=[], outs=[])
                        nop.engine = inst.engine
                        nop.sync_info = mybir.SyncInfo(on_wait=[w], on_update=[])
                        out.append(nop)
                    si.on_wait = waits[-cap:]
                    changed = True
                out.append(inst)
            if changed:
                bb.instructions = out


def _build_bass():
    import concourse.bass as bass
    import concourse.mybir as mybir
    from concourse import tile
    from contextlib import ExitStack

    f16 = mybir.dt.float16
    f32 = mybir.dt.float32
    AF = mybir.ActivationFunctionType
    OP = mybir.AluOpType

    nc = bass.Bass("TRN2", target_bir_lowering=False, debug=False)

    xc_d = nc.dram_tensor("xc", [C, PB, 128, W], f16, kind="ExternalInput").ap()
    yc_d = nc.dram_tensor("yc", [C, PB, 128, W], f16, kind="ExternalInput").ap()
    xx_d = nc.dram_tensor("xx", [C, PB, 128, W], f16, kind="ExternalInput").ap()
    xcbq_d = nc.dram_tensor("xcbq", [C, PB, 128, W], f16, kind="ExternalInput").ap()
    wq_d = nc.dram_tensor("wq", [128, 416], f16, kind="ExternalInput").ap()
    qv_d = nc.dram_tensor("qv", [128, PB], f32, kind="ExternalInput").ap()
    out_d = nc.dram_tensor("out", [C, PB, 128, W], f16, kind="ExternalOutput").ap()

    with tile.TileContext(nc) as tc, ExitStack() as ctx:
        pconst = ctx.enter_context(tc.tile_pool(name="const", bufs=1))
        wq_t = pconst.tile([128, 416], f16, tag="wq")
        nc.sync.dma_start(wq_t[:], wq_d[:])
        qv_t = pconst.tile([128, PB], f32, tag="qv")
        nc.sync.dma_start(qv_t[:], qv_d[:])

        def wslice(wt, a, b):
            if wt == "e0":
                return wq_t[:, a:b]
            if wt == "int":
                return wq_t[:, 136 + a : 136 + b]
            return wq_t[:, 280 + a : 280 + b]

        # ---- pools ----
        pio = ctx.enter_context(tc.tile_pool(name="io16", bufs=2))  # xc, yc
        pxy = ctx.enter_context(tc.tile_pool(name="xy", bufs=1))
        pxx = ctx.enter_context(tc.tile_pool(name="xx", bufs=1))
        pmid = ctx.enter_context(tc.tile_pool(name="midB", bufs=5))  # mids+sA/sB
        ppad = ctx.enter_context(tc.tile_pool(name="pad", bufs=1))
        prw = ctx.enter_context(tc.tile_pool(name="ring", bufs=2))
        prw1 = ctx.enter_context(tc.tile_pool(name="ring1", bufs=1))
        pxq = ctx.enter_context(tc.tile_pool(name="xq", bufs=1))
        pout = ctx.enter_context(tc.tile_pool(name="outst", bufs=2))
        pz_h = ctx.enter_context(tc.tile_pool(name="zh", bufs=2, space="PSUM"))
        pz_w = ctx.enter_context(tc.tile_pool(name="zw", bufs=2, space="PSUM"))

        def mm_group_full(z, lhs_of):
            mms = []
            for hf in (0, 1):
                first_in_bank = True
                for c, s, e, wt, wa, wb in _HALVES[hf]:
                    mms.append(
                        (z[:, s:e], lhs_of(c), wslice(wt, wa, wb), first_in_bank)
                    )
                    first_in_bank = False
            for i, (o, l, r, st) in enumerate(mms):
                nc.tensor.matmul(
                    o, l, r,
                    start=st,
                    stop=(i == len(mms) - 1),
                    skip_group_check=True,
                )
            return z

        # scan-input pad buffers: allocated once; zero pads memset once
        av_pad = ppad.tile([128, PB * CH_PAD], f16, tag="av_pad")
        bv_pad = ppad.tile([128, PB * CH_PAD], f16, tag="bv_pad")
        for buf in (av_pad, bv_pad):
            for c in range(PB):
                base = c * CH_PAD
                nc.gpsimd.memset(buf[:, base : base + PAD0], 0.0)
                nc.gpsimd.memset(buf[:, base + PAD0 + W : base + CH_PAD], 0.0)

        def emit_ph2(pch, sA, sB):
            for m in range(PB):
                z2a = pz_w.tile([128, W], f32, tag="zw")
                mm_group_full(
                    z2a,
                    lambda c, _m=m, _s=sA: _s[:, c * CH_S + 8 + 128 * _m : c * CH_S + 8 + 128 * _m + 128],
                )
                z2b = pz_w.tile([128, W], f32, tag="zw")
                mm_group_full(
                    z2b,
                    lambda c, _m=m, _s=sB: _s[:, c * CH_S + 8 + 128 * _m : c * CH_S + 8 + 128 * _m + 128],
                )
                xq = pxq.tile([128, W], f16, tag="xq")
                nc.sync.dma_start(xq[:], xcbq_d[pch, m])
                s2a = prw1.tile([128, W], f16, tag="f1")
                nc.scalar.activation(s2a[:], z2a[:], AF.Copy)
                s2b = prw1.tile([128, W], f16, tag="s2b")
                nc.scalar.activation(
                    s2b[:], z2b[:], AF.Copy, scale=qv_t[:, m : m + 1]
                )
                nc.vector.tensor_mul(s2a[:], s2a[:], xq[:])
                ot = pout.tile([128, W], f16, tag="outst")
                nc.vector.tensor_add(ot[:], s2a[:], s2b[:])
                nc.sync.dma_start(out_d[pch, m], ot[:])

        prev_ph2 = None
        for ch in range(C):
            # ---- stage 0: load inputs, make xy on DVE ----
            xc_big = pio.tile([128, PB * W], f16, tag="io16")
            yc_big = pio.tile([128, PB * W], f16, tag="io16")
            xx_big = pxx.tile([128, PB * W], f16, tag="xx")
            for b in range(PB):
                nc.sync.dma_start(xc_big[:, b * W : (b + 1) * W], xc_d[ch, b])
            xy_big = pxy.tile([128, PB * W], f16, tag="xy")
            for b2 in range(0, PB, 2):
                for b in (b2, b2 + 1):
                    nc.sync.dma_start(yc_big[:, b * W : (b + 1) * W], yc_d[ch, b])
                sl = slice(b2 * W, (b2 + 2) * W)
                nc.gpsimd.tensor_mul(xy_big[:, sl], xc_big[:, sl], yc_big[:, sl])
            for b in range(PB):
                nc.sync.dma_start(xx_big[:, b * W : (b + 1) * W], xx_d[ch, b])
            if prev_ph2 is not None:
                emit_ph2(prev_ph2[0], prev_ph2[1], prev_ph2[2])
                prev_ph2 = None

            # ---- stage 1a: H-matmul (A->B, qh folded); ACT evacs ----
            mids = {}
            for t, big, ev in (
                ("x", xc_big, "A"), ("y", yc_big, "D"),
                ("xy", xy_big, "A"), ("xx", xx_big, "D"),
            ):
                midt = pmid.tile([128, PB * CH_S], f16, tag="midB")
                for m in range(PB):
                    zh = pz_h.tile([128, W], f32, tag="zh")
                    mm_group_full(
                        zh,
                        lambda c, _b=big, _m=m: _b[:, c * W + 128 * _m : c * W + 128 * _m + 128],
                    )
                    dst = midt[:, m * CH_S : m * CH_S + W]
                    if ev == "A":
                        nc.scalar.activation(dst, zh[:], AF.Copy)
                    else:
                        nc.vector.tensor_copy(dst, zh[:])
                mids[t] = midt

            # ---- stage 1b: W-matmul (B->A, qw folded) + pointwise ----
            sA = pmid.tile([128, PB * CH_S], f16, tag="midB")
            sB = pmid.tile([128, PB * CH_S], f16, tag="midB")
            for hc in range(PB):
                def wmm(t, _hc=hc):
                    z = pz_w.tile([128, W], f32, tag="zw")
                    mm_group_full(
                        z,
                        lambda m, _t=t: mids[_t][:, m * CH_S + 128 * _hc : m * CH_S + 128 * _hc + 128],
                    )
                    return z

                z_x = wmm("x")
                mx = prw.tile([128, W], f16, tag="mx")
                nc.scalar.activation(mx[:], z_x[:], AF.Copy)
                z_y = wmm("y")
                my = prw.tile([128, W], f16, tag="my")
                nc.scalar.activation(my[:], z_y[:], AF.Copy)
                t1 = prw1.tile([128, W], f16, tag="t1")
                nc.vector.tensor_mul(t1[:], mx[:], my[:])
                z_xy = wmm("xy")
                num = prw1.tile([128, W], f16, tag="num")
                nc.vector.tensor_sub(num[:], z_xy[:], t1[:])
                z_xx = wmm("xx")
                u = prw1.tile([128, W], f16, tag="u")
                nc.scalar.activation(u[:], z_xx[:], AF.Copy, bias=EPS)
                s2 = prw1.tile([128, W], f16, tag="s2")
                nc.scalar.activation(s2[:], mx[:], AF.Square)
                nc.vector.tensor_sub(u[:], u[:], s2[:])  # u <- den
                with nc.allow_low_precision(
                    reason="18-bit reciprocal ample for eps-regularized den"
                ):
                    nc.vector.reciprocal(u[:], u[:])  # u <- 1/den
                avc = av_pad[:, hc * CH_PAD + PAD0 : hc * CH_PAD + PAD0 + W]
                nc.vector.tensor_mul(avc, num[:], u[:])
                nc.vector.tensor_mul(mx[:], avc, mx[:])  # mx <- j1 = a*mx
                # bv = (my + 0.5) - j1  (the 0.5 output shift rides through)
                bvc = bv_pad[:, hc * CH_PAD + PAD0 : hc * CH_PAD + PAD0 + W]
                nc.gpsimd.scalar_tensor_tensor(
                    bvc, my[:], 0.5, mx[:], OP.add, OP.subtract
                )
                base = hc * CH_PAD
                nc.gpsimd.tensor_tensor_scan(
                    sA[:, hc * CH_S : (hc + 1) * CH_S],
                    av_pad[:, base + PAD0 : base + PAD0 + SCAN_LEN],
                    av_pad[:, base + 1 : base + 1 + SCAN_LEN],
                    0.0,
                    OP.add,
                    OP.subtract,
                )
                nc.gpsimd.tensor_tensor_scan(
                    sB[:, hc * CH_S : (hc + 1) * CH_S],
                    bv_pad[:, base + PAD0 : base + PAD0 + SCAN_LEN],
                    bv_pad[:, base + 1 : base + 1 + SCAN_LEN],
                    0.0,
                    OP.add,
                    OP.subtract,
                )

            prev_ph2 = (ch, sA, sB)
        emit_ph2(prev_ph2[0], prev_ph2[1], prev_ph2[2])

    _split_multi_waits(nc, mybir)
    return nc


def _get_bass():
    if "nc" not in _CACHE:
        _CACHE["nc"] = _build_bass()
    return _CACHE["nc"]


def kernel(x, y):
    x = np.asarray(x)
    y = np.asarray(y)
    from concourse.bass_utils import run_bass_kernel_spmd

    nc = _get_bass()
    wq, qv = _host_consts()
    B = x.shape[0]
    xcf = (x - 0.5).astype(np.float16)
    ycf = (y - 0.5).astype(np.float16)
    xxf = (xcf.astype(np.float32) ** 2).astype(np.float16)
    qw_w = (1.0 / _counts()).astype(np.float32)  # per-W normalization
    xcbq = np.ascontiguousarray(xcf.transpose(0, 1, 3, 2).astype(np.float32)
                                * qw_w[None, None, :, None]).astype(np.float16)
    xc = xcf.reshape(B, C, PB, 128, W)
    yc = ycf.reshape(B, C, PB, 128, W)
    xx = xxf.reshape(B, C, PB, 128, W)
    xcbq = xcbq.reshape(B, C, PB, 128, W)
    in_maps = [
        {"xc": xc[i], "yc": yc[i], "xx": xx[i], "xcbq": xcbq[i], "wq": wq, "qv": qv}
        for i in range(B)
    ]
    res = run_bass_kernel_spmd(nc, in_maps, core_ids=list(range(B)))
    out = np.stack(
        [
            res.results[i]["out"].reshape(C, W, H).transpose(0, 2, 1)
            for i in range(B)
        ]
    )
    return np.ascontiguousarray(out).astype(np.float32)
